# revision 49
# baseline (speedup 1.0000x reference)
"""Trainium2 Bass kernel for nn_Auto_Attn (self-attention + context flow + convs).

Sharding: 8 cores = (batch b in 0..3) x (image half s in 0..1). Each core
computes its half's 32 output rows plus 2 halo rows of the attention output
(conv1 3x3 -> conv2 3x3 needs a 2-row inp halo).

Per-core algorithm (validated against a float64 reference in numpy):
  q = wq @ x (1x1 conv);  energy[m,n] = q_m . q_n  (symmetric since key==query)
  Softmax rows are computed TRANSPOSED (eT[n,m], n on partitions) so the
  attention einsums need no transposes of the big attn matrix:
    - shift-invariance: softmax(e[m,:]) == softmax(e[m,:] - d[m]) for
      d[m] = e[m,m]; Cauchy-Schwarz bounds e[m,n]-d[m] <= (e[n,n]-e[m,m])/2
      so exp never overflows fp32. The shift is folded into the energy matmul
      via an augmented contraction row: q_aug_n = [q; 1], q_aug_m = [q; -d].
    - Z[m] arrives free as a 257th ones-column in the value matrix.

Attention stream: one loop over (m-chunk, nt-quad) steps — m in 128-wide
chunks, n in 128-tiles.  The softmax weights are produced DIRECTLY IN FP8:
wq is host-scaled by sqrt(K8), K8 = 8/ln2, and the augmented row holds
-K8*d[m] + B, so the energy PSUM holds the fp8e4 *bit value* of
S*exp(e-d), S = 2^((B-56)/8).  Two exp flavors, split by m-chunk so any
flavor-systematic factor cancels in 1/Z: a DVE bit-trick ((bits-0.458)
max 0 -> u8, reinterpreted as fp8) and an ACT true Exp with fp8 output.
B = 100 keeps the empirical max energy diff (0.883 ln) ~8 fp8-exponent
bits under the 240 overflow bound.

The value matmuls then run fp8 x fp8 in DoubleRow perf mode (two n-tiles
contracted per instruction, 2 cols/cycle): expT pairs are the stationary
operand, the value matrix blocks are padded to VPTP=528 so the pair step
satisfies the %16==0 ISA constraint.  ~1.7x over the bf16 path.

Convs: 1D row-Winograd F(2,3).  Host sends G-transformed weights (4
row-phases x 3 col-taps); the device forms T0..T3 per 2-row strip with
4 DVE ops per cin chunk, accumulates 4 phase PSUMs (the 1x1 shortcut is
folded into M0/M3 with a host-negated copy), and combines y0 = M0+M1+M2,
y1 = M1-M2-M3 on DVE (M1 staged via SBUF: one-PSUM-operand rule).  1.5x
fewer PE MACs than direct 3x3; conv1's lrelu runs during the attention
tail as inp rows finalize.

DMA: all large tensors are host-packed into their exact SBUF layout; the
two hwdge queues run in parallel (SP: xb lower half + epilogue windows,
ACT: xb upper half, the fp8 value matrix, then the conv weights).  The
q-aug scalars are computed per 512-chunk so the energy stream starts as
soon as chunk 0 lands.

dtypes: bf16 q/energy path, fp8e4 softmax weights + value matrix
(DoubleRow), bf16 Winograd convs; f32 PSUM accumulation throughout.
"""
import numpy as np
from contextlib import ExitStack

import ml_dtypes
import concourse.bass as bass
import concourse.tile as tile
from concourse import bacc, mybir
from concourse.bass_utils import run_bass_kernel_spmd
from concourse.masks import make_identity

F32 = mybir.dt.float32
F32R = mybir.dt.float32r
BF16 = mybir.dt.bfloat16
FP16 = mybir.dt.float16
F8E4 = mybir.dt.float8e4
U8 = mybir.dt.uint8

B, C, H, W = 4, 256, 64, 64
N = H * W                # 4096
CQ = 64
K8 = 8.0 / np.log(2.0)   # fp8e4 bits per ln-unit (11.5416)
BEXP = 100.0             # bits bias: diag -> 2^5.5, empirical max diff 0.883 ln
                         # -> max bits ~110.2, fp8e4 max normal = bits 119
HALF = 32
WIN = 36                 # window rows (32 + 2 halo each side)
BUFR, BUFC = 38, 66      # padded conv buffer: +1 guard row / +1 pad col each side
MW = WIN * 64            # 2304 m positions per core
NT = N // 128            # 32 n-tiles
NPB = NT // 2            # nt-pairs per m-chunk
MBLK = 128               # attention m-chunk (2 image rows)
NBLK = MW // MBLK        # 18
VPT = 513                # [v(256) | ones(1) | pre(256)]
VPTP = 528               # padded block stride: DoubleRow pair step must be %16==0
POS1_LO, POS1_HI = 2 * BUFC, 36 * BUFC     # conv1 output span (2244)
POS2_LO, POS2_HI = 3 * BUFC, 35 * BUFC     # conv2/out span (2112)
C1_CHUNKS = [512, 512, 512, 452, 256]      # sum 2244
C2_CHUNKS = [512, 512, 512, 320, 256]      # sum 2112

_PROGRAM_CACHE = {}


def tap_off(t):
    return (t // 3 - 1) * BUFC + (t % 3 - 1)


# --------------------------------------------------------------------------
# host-side prep
# --------------------------------------------------------------------------

def host_prep(inputs, core):
    b, s = core // 2, core % 2
    f32 = np.float32
    bf16 = ml_dtypes.bfloat16
    x = np.asarray(inputs['x'][b], f32).reshape(C, N)
    pre = np.asarray(inputs['pre'][b], f32).reshape(C, N)
    M = np.asarray(inputs['mask'][b, 0], f32).reshape(N)
    gamma = float(np.asarray(inputs['gamma']).reshape(-1)[0])
    alpha = float(np.asarray(inputs['alpha']).reshape(-1)[0])

    r_lo = s * HALF - 2
    rows = np.arange(r_lo, r_lo + WIN)
    valid_rows = (rows >= 0) & (rows < H)
    valid = np.repeat(valid_rows, 64).astype(f32)

    def win_slice(t):
        o = np.zeros((t.shape[0], MW), f32)
        vr = np.where(valid_rows)[0]
        o[:, vr[0] * 64:vr[-1] * 64 + 64] = t[:, rows[vr[0]] * 64: rows[vr[-1]] * 64 + 64]
        return o

    xm = win_slice(x)
    pm = win_slice(pre)
    Mw = win_slice(M[None, :])[0]

    # rotate the n-axis so window position p is original column p-128 (s=0)
    # / p+1920 (s=1): q_m then slices directly out of q_n on device.  The
    # value-matrix rows carry the same rotation, so the softmax contraction
    # is unchanged.
    roll = 128 if s == 0 else -1920
    vpT = np.zeros((N, VPTP), f32)
    vpT[:, 0:C] = x.T
    vpT[:, C] = 1.0
    vpT[:, C + 1:VPT] = pre.T
    vpT = np.roll(vpT, roll, axis=0)

    def sb(a, inner):  # [K*128, inner] -> SBUF layout [128, K*inner]
        k = a.shape[0] // 128
        return np.ascontiguousarray(
            a.reshape(k, 128, inner).transpose(1, 0, 2).reshape(128, k * inner))

    def pervec(v):  # [MW] -> [128, 18] (per-partition layout per m-chunk)
        return np.ascontiguousarray(v.reshape(NBLK, 128).T)

    wq = np.asarray(inputs['wq'], f32).reshape(CQ, C)
    w1 = np.asarray(inputs['w1'], f32)
    w2 = np.asarray(inputs['w2'], f32)
    ws = np.asarray(inputs['ws'], f32).reshape(C, 2 * C)
    bq = np.asarray(inputs['bq'], f32).reshape(CQ, 1)
    b1 = np.asarray(inputs['b1'], f32)
    b2 = np.asarray(inputs['b2'], f32)
    bs = np.asarray(inputs['bs'], f32)

    # 1D row-Winograd F(2,3) weights: W'[p,kw,i,o] = sum_kh G[p,kh] w[o,i,kh,kw]
    # SBUF layout [128, (p, kw, in_chunk)*C]
    GW = np.array([[1, 0, 0], [.5, .5, .5], [.5, -.5, .5], [0, 0, 1]], f32)
    w1p = np.einsum('ph,oihw->pwio', GW, w1)
    w1s = np.ascontiguousarray(
        w1p.reshape(4, 3, 4, 128, C).transpose(3, 0, 1, 2, 4).reshape(128, 48 * C))
    w2p = np.einsum('ph,oihw->pwio', GW, w2)
    w2s = np.ascontiguousarray(
        w2p.reshape(4, 3, 2, 128, C).transpose(3, 0, 1, 2, 4).reshape(128, 24 * C))
    # shortcut weights, plus a negated copy (folded into Winograd phase M3,
    # which enters y1 with a minus sign)
    wsT = ws.T.copy()
    wss = np.concatenate([sb(wsT, C), sb(-wsT, C)], axis=1)

    br_rows = r_lo + np.arange(BUFR) - 1
    rv = ((br_rows >= 0) & (br_rows < H)).astype(f32)
    cv = np.zeros(BUFC, f32)
    cv[1:65] = 1.0
    hrmask = np.broadcast_to((rv[:, None] * cv[None, :]).reshape(1, -1),
                             (128, BUFR * BUFC))

    return {
        'xb': sb(np.roll(x, roll, axis=1), N).astype(bf16),
        'xm': sb(xm, MW).astype(bf16),
        'pm2': sb((1.0 - Mw) * valid * pm, MW).astype(bf16),
        'vpT': sb(vpT.reshape(NT * 128, VPTP), VPTP).astype(ml_dtypes.float8_e4m3),
        'avec': pervec(gamma * valid),
        'b1vec': pervec(alpha * Mw * valid),
        'wq': sb(np.ascontiguousarray(wq.T) * np.sqrt(K8), CQ).astype(bf16),
        'bq': bq * np.sqrt(K8),
        'w1': w1s.astype(bf16),
        'w2': w2s.astype(bf16),
        'ws': wss.astype(bf16),
        'b1c': np.ascontiguousarray(b1.reshape(2, 128).T),
        'bfin': np.ascontiguousarray((b2 + bs).reshape(2, 128).T),
        'hrmask': np.ascontiguousarray(hrmask).astype(bf16),
    }


# --------------------------------------------------------------------------
# device program
# --------------------------------------------------------------------------

def build_program():
    nc = bacc.Bacc("TRN2", target_bir_lowering=False, debug=False, num_devices=8)

    def din(name, shape, dt=F32):
        return nc.dram_tensor(name, shape, dt, kind="ExternalInput").ap()

    xb_d = din('xb', [128, 2 * N], BF16)
    xm_d = din('xm', [128, 2 * MW], BF16)
    pm2_d = din('pm2', [128, 2 * MW], BF16)
    vpT_d = din('vpT', [128, NT * VPTP], F8E4)
    avec_d = din('avec', [128, NBLK])
    b1vec_d = din('b1vec', [128, NBLK])
    wq_d = din('wq', [128, 2 * CQ], BF16)
    bq_d = din('bq', [CQ, 1])
    w1_d = din('w1', [128, 48 * C], BF16)
    w2_d = din('w2', [128, 24 * C], BF16)
    ws_d = din('ws', [128, 8 * C], BF16)
    b1c_d = din('b1c', [128, 2])
    bfin_d = din('bfin', [128, 2])
    hrmask_d = din('hrmask', [128, BUFR * BUFC], BF16)
    y_d = nc.dram_tensor('y', [2, 128, POS2_HI - POS2_LO], BF16,
                         kind="ExternalOutput").ap()

    with tile.TileContext(nc) as tc, ExitStack() as ctx:
        # ---------------- persistent tiles ----------------
        persist = ctx.enter_context(tc.tile_pool(name="persist", bufs=1))
        inp_raw = [persist.tile([128, BUFR * BUFC], BF16, tag=f"inp{i}", name=f"inp{i}")
                   for i in range(4)]
        h1 = [persist.tile([128, BUFR * BUFC], BF16, tag=f"h1{i}", name=f"h1{i}")
              for i in range(2)]
        w1_t = persist.tile([128, 48 * C], BF16, tag="w1")
        w2_t = persist.tile([128, 24 * C], BF16, tag="w2")
        ws_t = persist.tile([128, 8 * C], BF16, tag="ws")
        b1c_t = persist.tile([128, 2], F32, tag="b1c")
        bfin_t = persist.tile([128, 2], F32, tag="bfin")
        hrm_t = persist.tile([128, BUFR * BUFC], BF16, tag="hrm")
        out_sb = [persist.tile([128, POS2_HI - POS2_LO], BF16, tag=f"os{i}", name=f"os{i}")
                  for i in range(2)]
        ident = persist.tile([128, 128], BF16, tag="id")
        # lrelu(inp) lives in persist so its Prelu ops can run during the
        # attention tail, as the epilogues finalize inp rows top-down.
        lrf = [persist.tile([128, BUFR * BUFC], BF16, tag=f"lrf{i}", name=f"lrf{i}")
               for i in range(4)]

        make_identity(nc, ident[:])
        # conv buffer border zeroing: cols 0 and 65 of every row (the only
        # positions the conv taps read that the attention epilogue doesn't
        # write).
        for t_ in inp_raw:
            a = t_[:].rearrange("p (r c) -> p r c", c=BUFC)
            nc.gpsimd.memset(a[:, :, 0:1], 0.0)
            nc.gpsimd.memset(a[:, :, 65:66], 0.0)

        def dma_weights():
            nc.scalar.dma_start(w1_t[:], w1_d)
            nc.scalar.dma_start(w2_t[:], w2_d)
            nc.scalar.dma_start(ws_t[:], ws_d)
            nc.scalar.dma_start(hrm_t[:], hrmask_d)
            nc.scalar.dma_start(b1c_t[:], b1c_d)
            nc.scalar.dma_start(bfin_t[:], bfin_d)

        # attention-lifetime tiles live in their own pool so the conv phase
        # can reuse the SBUF space after it closes.
        atp_cm = tc.tile_pool(name="attn", bufs=1)
        atp = atp_cm.__enter__()
        # vpT / q_aug_n are split into per-group tiles: DMA->read dependencies
        # are tile-granular, so a monolithic tile would stall the first value
        # matmul until the LAST group's DMA lands.
        vpt = [atp.tile([128, 4 * VPTP], F8E4, tag=f"vpT{g}", name=f"vpT{g}")
               for g in range(8)]
        xm_t = atp.tile([128, 2 * MW], BF16, tag="xm")
        pm2a = atp.tile([128, MW], BF16, tag="pm2a")
        pm2b = atp.tile([128, MW], BF16, tag="pm2b")
        avec_t = atp.tile([128, NBLK], F32, tag="av")
        b1vec_t = atp.tile([128, NBLK], F32, tag="b1v")
        wq_t = atp.tile([128, 2 * CQ], BF16, tag="wq")
        bq_t = atp.tile([CQ, 1], F32, tag="bq")
        ebias_t = atp.tile([128, 1], F32, tag="ebias")
        qn = [atp.tile([CQ + 1, 512], BF16, tag=f"qn{k}", name=f"qn{k}")
              for k in range(8)]
        # q_aug_m split into per-512-chunk tiles: the energy stream's early
        # quads unblock as soon as chunk 0's aug row lands (deps are
        # tile-granular), instead of waiting for the whole window.
        MCH = [512, 512, 512, 512, 256]
        qm = [atp.tile([CQ + 1, MCH[c]], BF16, tag=f"qm{c}", name=f"qm{c}")
              for c in range(5)]
        expT = atp.tile([128, NT * MBLK], F8E4, tag="expT")

        # Every DMA trigger costs ~600ns of sync-engine queue time, so the
        # q-path critical tensors (xm below, then wq/bq) go first; avec/b1vec
        # are not read until the first epilogue (~20us in) and follow the
        # value matrix.  The conv-phase constants are deferred into the
        # attention stream (ACT queue) so they don't steal early bandwidth.
        for k in range(8):
            nc.gpsimd.memset(qn[k][CQ:CQ + 1, :], 1.0)

        def dma_vpt_group(g):
            # value matrix rides the second hwdge queue (ACT), in parallel
            # with the q-path stream on SP
            nc.scalar.dma_start(vpt[g][:], vpT_d[:, g * 4 * VPTP:(g + 1) * 4 * VPTP])

        # ---------------- q phase ----------------
        with tc.tile_pool(name="qp", bufs=1) as qp, \
             tc.tile_pool(name="qps", bufs=2, space="PSUM") as qps:
            nc.sync.dma_start(wq_t[:], wq_d)
            nc.sync.dma_start(bq_t[:], bq_d)

            # q over the full image; xb is chunked so the lower half (the
            # first 16 n-tiles) lands first and the energy stream can start.
            # Per-chunk q_aug work (copy/square/d-sum/aug row) runs as soon
            # as its qn lands so the first energy quad unblocks ~15us in;
            # q_m is a plain slice of the (host-rotated) q_n (see host_prep).
            ones_t = qp.tile([CQ, 1], BF16, tag="ones")
            nc.vector.memset(ones_t[:], 1.0)

            def emit_mchunk(c):
                cs = MCH[c]
                qsq = qp.tile([CQ, 512], BF16, tag=f"qsq{c}", name=f"qsq{c}")
                nc.vector.tensor_copy(qm[c][0:CQ, 0:cs], qn[c][0:CQ, 0:cs])
                nc.vector.tensor_mul(qsq[:, 0:cs], qn[c][0:CQ, 0:cs],
                                     qn[c][0:CQ, 0:cs])
                dps = qps.tile([1, 512], F32, tag="dpsum")
                nc.tensor.matmul(dps[:, 0:cs], ones_t[:], qsq[:, 0:cs],
                                 start=True, stop=True)
                nc.vector.tensor_scalar(qm[c][CQ:CQ + 1, 0:cs],
                                        dps[:, 0:cs], -1.0, BEXP,
                                        mybir.AluOpType.mult,
                                        mybir.AluOpType.add)

            # xb halves ride both hwdge queues in parallel (the q chain is the
            # startup critical path); vpT follows on ACT behind xb's top half.
            xb4 = {}
            for hi in range(2):
                eng = nc.sync if hi == 0 else nc.scalar
                for ck in range(2):
                    xb4[hi, ck] = qp.tile([128, 2048], BF16, tag=f"xb{hi}{ck}",
                                          name=f"xb{hi}{ck}")
                    eng.dma_start(
                        xb4[hi, ck][:],
                        xb_d[:, ck * N + hi * 2048:ck * N + (hi + 1) * 2048])
            dma_vpt_group(0)
            dma_vpt_group(1)
            for hi in range(2):
                for nb in range(4 * hi, 4 * hi + 4):
                    qsum = qps.tile([CQ, 512], F32, tag="qpsum")
                    for ck in range(2):
                        nc.tensor.matmul(
                            qsum[:], wq_t[:, ck * CQ:(ck + 1) * CQ],
                            xb4[hi, ck][:, (nb % 4) * 512:(nb % 4 + 1) * 512],
                            start=(ck == 0), stop=(ck == 1))
                    nc.scalar.activation(qn[nb][0:CQ, :], qsum[:],
                                         mybir.ActivationFunctionType.Identity,
                                         bias=bq_t[:])
                    if nb <= 4:
                        emit_mchunk(nb)
            nc.sync.dma_start(xm_t[:], xm_d)
            nc.sync.dma_start(avec_t[:], avec_d)
            nc.sync.dma_start(b1vec_t[:], b1vec_d)
            for g in range(2, 8):
                dma_vpt_group(g)
            nc.sync.dma_start(pm2a[:], pm2_d[:, 0:MW])
            nc.sync.dma_start(pm2b[:], pm2_d[:, MW:2 * MW])

        # ---------------- attention: one stream over (m-chunk, nt-pair) ----
        with tc.tile_pool(name="eps", bufs=2, space="PSUM") as eps, \
             tc.tile_pool(name="pop", bufs=2, space="PSUM") as pop, \
             tc.tile_pool(name="tps", bufs=2, space="PSUM") as tps, \
             tc.tile_pool(name="epp", bufs=2) as epp:

            # PSUM energy already holds the fp8e4 bit value k*(e-d)+B (wq is
            # host-scaled by sqrt(K8), the aug row adds -k*d+B).  Two exp
            # flavors, split by m-chunk so any flavor-systematic factor is
            # row-uniform and cancels exactly in the 1/Z normalization:
            #  - DVE bit-trick: (bits - 0.458) max 0 -> u8 -> reinterpret fp8
            #    (-0.458 centers the mantissa-linear approx, +-2.9%)
            #  - ACT true exp: S*e^(e-d) = Exp(bits/K8 + lnS - B/K8), fp8 out
            SEXP = float(2.0 ** ((BEXP - 56.0) / 8.0))
            EBIAS = float(np.log(SEXP) - BEXP / K8)
            nc.gpsimd.memset(ebias_t[:], EBIAS)

            def emit_energy_quad(blk, k):
                mo = (blk % 4) * 128
                ets = eps.tile([128, 4 * MBLK], F32, tag="ets")
                for i in range(4):
                    nc.tensor.matmul(ets[:, i * MBLK:(i + 1) * MBLK],
                                     qn[k][:, i * 128:(i + 1) * 128],
                                     qm[blk // 4][:, mo:mo + MBLK],
                                     start=(i == 0), stop=(i == 3),
                                     skip_group_check=True)
                dst = expT[:, 4 * k * MBLK:(4 * k + 4) * MBLK]
                if blk % 3 == 2:
                    nc.vector.tensor_scalar(dst.bitcast(U8), ets[:],
                                            -0.458, 0.0,
                                            mybir.AluOpType.add,
                                            mybir.AluOpType.max)
                else:
                    nc.scalar.activation(dst, ets[:],
                                         mybir.ActivationFunctionType.Exp,
                                         bias=ebias_t[:], scale=float(1.0 / K8))

            def emit_value(po, pr):
                # DoubleRow fp8 pair: nt = 2*pr, 2*pr+1 in one instruction
                lhs = expT[:, pr * 2 * MBLK:(pr + 1) * 2 * MBLK].rearrange(
                    "p (two m) -> p two m", two=2)
                g, j = pr // 2, pr % 2
                rhs = vpt[g][:, j * 2 * VPTP:(j + 1) * 2 * VPTP].rearrange(
                    "p (two v) -> p two v", two=2)
                nc.tensor.matmul(po[0][:], lhs, rhs[:, :, 0:257],
                                 start=(pr == 0), stop=(pr == NT // 2 - 1),
                                 perf_mode=mybir.MatmulPerfMode.DoubleRow)
                nc.tensor.matmul(po[1][:], lhs, rhs[:, :, 257:513],
                                 start=(pr == 0), stop=(pr == NT // 2 - 1),
                                 perf_mode=mybir.MatmulPerfMode.DoubleRow)

            def emit_epi_scalings(po, j):
                po_a, po_b = po
                rt = epp.tile([128, 1], F32, tag="rt")
                s1 = epp.tile([128, 1], F32, tag="s1")
                s2 = epp.tile([128, 1], F32, tag="s2")
                nc.vector.reciprocal(rt[:], po_a[:, 256:257])
                nc.vector.tensor_mul(s1[:], rt[:], avec_t[:, j:j + 1])
                nc.vector.tensor_mul(s2[:], rt[:], b1vec_t[:, j:j + 1])
                ov = epp.tile([128, C], BF16, tag="ov")
                cv = epp.tile([128, C], BF16, tag="cv")
                nc.vector.tensor_scalar_mul(ov[:], po_a[:, 0:C], s1[:])
                nc.vector.tensor_scalar_mul(cv[:], po_b[:, 0:C], s2[:])
                return (ov, cv, j)

            def emit_epi_half(pend, half):
                ov, cv, j = pend
                br0 = 1 + 2 * j
                for h_ in (2 * half, 2 * half + 1):
                    src = (ov if h_ < 2 else cv)
                    ck = h_ % 2
                    if h_ < 2:
                        add_ap = xm_t[:, ck * MW + j * 128:ck * MW + (j + 1) * 128]
                    else:
                        add_ap = (pm2a if ck == 0 else pm2b)[:, j * 128:(j + 1) * 128]
                    pt = tps.tile([128, 128], BF16, tag="pt")
                    nc.tensor.transpose(pt[:], src[:, ck * 128:ck * 128 + 128],
                                        ident[:])
                    dst = inp_raw[h_][:].rearrange("p (r c) -> p r c", c=BUFC)
                    nc.vector.tensor_add(
                        dst[:, br0:br0 + 2, 1:65],
                        pt[:].rearrange("p (r c) -> p r c", c=64),
                        add_ap.rearrange("p (r c) -> p r c", c=64))

            quads = [(blk, k) for blk in range(NBLK) for k in range(NT // 4)]
            po_of = {}

            # conv1's lrelu(inp) runs during the attention tail: rows finalize
            # top-down as epilogues land (epi j writes rows 1+2j, 2+2j).
            LRX = {(9, kk): (kk - 2, 1, 17) for kk in range(2, 6)}
            LRX.update({(13, kk): (kk - 2, 17, 25) for kk in range(2, 6)})
            LRX.update({(17, kk): (kk - 2, 25, 33) for kk in range(2, 6)})

            def get_po(blk):
                if blk not in po_of:
                    po_of[blk] = (pop.tile([128, 257], F32, tag="poa", name="poa"),
                                  pop.tile([128, 256], F32, tag="pob", name="pob"))
                return po_of[blk]

            emit_energy_quad(*quads[0])
            emit_energy_quad(*quads[1])
            pend = None
            for g, (blk, k) in enumerate(quads):
                po = get_po(blk)
                if k == 0 and blk > 0:
                    pend = emit_epi_scalings(get_po(blk - 1), blk - 1)
                for j in range(2):
                    emit_value(po, 2 * k + j)
                if g + 2 < len(quads):
                    emit_energy_quad(*quads[g + 2])
                if g == 96:
                    dma_weights()
                if (blk, k) in LRX:
                    i_, r0, r1 = LRX[(blk, k)]
                    nc.scalar.activation(lrf[i_][:, r0 * BUFC:r1 * BUFC],
                                         inp_raw[i_][:, r0 * BUFC:r1 * BUFC],
                                         mybir.ActivationFunctionType.Prelu,
                                         alpha=0.1)
                if pend is not None and k == 1:
                    emit_epi_half(pend, 0)
                elif pend is not None and k == 2:
                    emit_epi_half(pend, 1)
                    pend = None
            pend = emit_epi_scalings(get_po(NBLK - 1), NBLK - 1)
            emit_epi_half(pend, 0)
            emit_epi_half(pend, 1)

        atp_cm.__exit__(None, None, None)

        # ---------------- conv phase: 1D row-Winograd F(2,3) ----------------
        # Per 2-output-row strip: 4 row-phases T0..T3 (input row transform),
        # phase conv = 3 col-taps x cin matmuls each, outputs y0 = M0+M1+M2,
        # y1 = M1-M2-M3 (ws shortcut folded: +ws.inp[r0] into M0, -ws.inp[r1]
        # into M3).  1.5x fewer PE MACs than direct 3x3.
        # T layouts carry a zero guard col between strip-GROUPS (not just the
        # ends) so a group's tap-shifted matmul reads never overlap the next
        # group's transform writes (which would serialize the interleave).
        GO1 = (1, 464, 927)     # conv1 group offsets (7, 7, 3 strips)
        T1W = 1126
        G1GUARD = (0, 463, 926, 1125)
        GO2 = (1, 464, 927)     # conv2 group offsets (7, 7, 2 strips)
        T2W = 1060
        G2GUARD = (0, 463, 926, 1059)
        with tc.tile_pool(name="cvp", bufs=1) as cvp, \
             tc.tile_pool(name="cts", bufs=2) as cts, \
             tc.tile_pool(name="cps", bufs=2, space="PSUM") as cps:
            T1 = [[cvp.tile([128, T1W], BF16, tag=f"T1_{i}_{c}", name=f"T1_{i}_{c}")
                   for c in range(4)] for i in range(4)]
            T2 = [[cvp.tile([128, T2W], BF16, tag=f"T2_{i}_{c}", name=f"T2_{i}_{c}")
                   for c in range(2)] for i in range(4)]
            for tt, gds in [(t, G1GUARD) for r in T1 for t in r] + \
                           [(t, G2GUARD) for r in T2 for t in r]:
                for gc in gds:
                    nc.gpsimd.memset(tt[:, gc:gc + 1], 0.0)

            def prs(tile_):
                return tile_[:].rearrange("p (r2 c2) -> p r2 c2", c2=2 * BUFC)

            # lrelu of the remaining inp rows (33..36; 1..32 ran in-stream)
            for i in range(4):
                nc.scalar.activation(lrf[i][:, 33 * BUFC:37 * BUFC],
                                     inp_raw[i][:, 33 * BUFC:37 * BUFC],
                                     mybir.ActivationFunctionType.Prelu,
                                     alpha=0.1)

            # conv1 input row transform: strip a -> input rows 1+2a..4+2a.
            # Emitted per strip-group so group-0 matmuls start after ~3us of
            # transforms and later groups' transforms hide under PE work.
            def t1_group(gi, a0, nst):
                for c in range(4):
                    pv = prs(lrf[c])
                    r1 = pv[:, a0:a0 + nst, 66:132]
                    r2 = pv[:, 1 + a0:1 + a0 + nst, 0:66]
                    r3 = pv[:, 1 + a0:1 + a0 + nst, 66:132]
                    r4 = pv[:, 2 + a0:2 + a0 + nst, 0:66]
                    dst = [T1[i][c][:, GO1[gi]:GO1[gi] + nst * 66].rearrange(
                        "p (s c) -> p s c", c=66) for i in range(4)]
                    nc.vector.tensor_sub(dst[0], r1, r3)
                    nc.vector.tensor_add(dst[1], r2, r3)
                    nc.vector.tensor_sub(dst[2], r3, r2)
                    nc.vector.tensor_sub(dst[3], r2, r4)

            def wino_out(Mp, wdt, dst0, dst1, func, bias_ap, alpha):
                # DVE may read only one PSUM operand per op: stage M1 in SBUF.
                s1 = cts.tile([128, 462], F32, tag="s1w")
                u0 = cts.tile([128, 462], F32, tag="u0")
                u1 = cts.tile([128, 462], F32, tag="u1")
                y0 = cts.tile([128, 462], F32, tag="y0")
                y1 = cts.tile([128, 462], F32, tag="y1")
                nc.vector.tensor_copy(s1[:, 0:wdt], Mp[1][:, 0:wdt])
                nc.vector.tensor_add(u0[:, 0:wdt], s1[:, 0:wdt], Mp[0][:, 0:wdt])
                nc.vector.tensor_add(y0[:, 0:wdt], u0[:, 0:wdt], Mp[2][:, 0:wdt])
                nc.vector.tensor_sub(u1[:, 0:wdt], s1[:, 0:wdt], Mp[2][:, 0:wdt])
                nc.vector.tensor_sub(y1[:, 0:wdt], u1[:, 0:wdt], Mp[3][:, 0:wdt])
                nc.scalar.activation(dst0, y0[:, 0:wdt].rearrange(
                    "p (s c) -> p s c", c=66), func, bias=bias_ap, alpha=alpha)
                nc.scalar.activation(dst1, y1[:, 0:wdt].rearrange(
                    "p (s c) -> p s c", c=66), func, bias=bias_ap, alpha=alpha)

            # conv1 phase matmuls + output transform -> h1 (lrelu'd + masked)
            C1G = [(0, 7), (7, 7), (14, 3)]
            for gi, (a0, nst) in enumerate(C1G):
                t1_group(gi, a0, nst)
                wdt = nst * 66
                for oc in range(2):
                    Mp = [cps.tile([128, 462], F32, tag=f"mp{i}", name=f"mp{i}")
                          for i in range(4)]
                    for i in range(4):
                        kk = 0
                        for t in range(3):
                            for cin in range(4):
                                bi = (i * 3 + t) * 4 + cin
                                nc.tensor.matmul(
                                    Mp[i][:, 0:wdt],
                                    w1_t[:, bi * C + oc * 128:bi * C + oc * 128 + 128],
                                    T1[i][cin][:, GO1[gi] + t - 1:GO1[gi] + t - 1 + wdt],
                                    start=(kk == 0), stop=(kk == 11))
                                kk += 1
                    hv = prs(h1[oc])
                    mv = prs(hrm_t)
                    d0 = hv[:, 1 + a0:1 + a0 + nst, 0:66]
                    d1 = hv[:, 1 + a0:1 + a0 + nst, 66:132]
                    wino_out(Mp, wdt, d0, d1,
                             mybir.ActivationFunctionType.Prelu,
                             b1c_t[:, oc:oc + 1], 0.1)
                    nc.vector.tensor_mul(d0, d0, mv[:, 1 + a0:1 + a0 + nst, 0:66])
                    nc.vector.tensor_mul(d1, d1, mv[:, 1 + a0:1 + a0 + nst, 66:132])

            # conv2 input row transform: strip a -> h1 rows 2+2a..5+2a
            def t2_group(gi, a0, nst):
                for c in range(2):
                    pv = prs(h1[c])
                    r1 = pv[:, 1 + a0:1 + a0 + nst, 0:66]
                    r2 = pv[:, 1 + a0:1 + a0 + nst, 66:132]
                    r3 = pv[:, 2 + a0:2 + a0 + nst, 0:66]
                    r4 = pv[:, 2 + a0:2 + a0 + nst, 66:132]
                    dst = [T2[i][c][:, GO2[gi]:GO2[gi] + nst * 66].rearrange(
                        "p (s c) -> p s c", c=66) for i in range(4)]
                    nc.vector.tensor_sub(dst[0], r1, r3)
                    nc.vector.tensor_add(dst[1], r2, r3)
                    nc.vector.tensor_sub(dst[2], r3, r2)
                    nc.vector.tensor_sub(dst[3], r2, r4)

            # conv2 phase matmuls (+ shortcut folded) -> out
            for gi, (a0, nst) in enumerate([(0, 7), (7, 7), (14, 2)]):
                t2_group(gi, a0, nst)
                wdt = nst * 66
                for oc in range(2):
                    Mp = [cps.tile([128, 462], F32, tag=f"mp{i}", name=f"mp{i}")
                          for i in range(4)]
                    iv = [prs(inp_raw[cin]) for cin in range(4)]
                    for i in range(4):
                        mms = []
                        for t in range(3):
                            for cin in range(2):
                                bi = (i * 3 + t) * 2 + cin
                                mms.append((
                                    w2_t[:, bi * C + oc * 128:bi * C + oc * 128 + 128],
                                    T2[i][cin][:, GO2[gi] + t - 1:GO2[gi] + t - 1 + wdt]))
                        if i == 0:      # + ws . inp[row 3+2a]
                            for cin in range(4):
                                mms.append((
                                    ws_t[:, cin * C + oc * 128:cin * C + oc * 128 + 128],
                                    iv[cin][:, 1 + a0:1 + a0 + nst, 66:132]))
                        if i == 3:      # - ws . inp[row 4+2a]  (negated copy)
                            for cin in range(4):
                                mms.append((
                                    ws_t[:, (4 + cin) * C + oc * 128:(4 + cin) * C + oc * 128 + 128],
                                    iv[cin][:, 2 + a0:2 + a0 + nst, 0:66]))
                        for kk, (wap, rap) in enumerate(mms):
                            nc.tensor.matmul(Mp[i][:, 0:wdt], wap, rap,
                                             start=(kk == 0),
                                             stop=(kk == len(mms) - 1))
                    ov = prs(out_sb[oc])
                    d0 = ov[:, a0:a0 + nst, 0:66]
                    d1 = ov[:, a0:a0 + nst, 66:132]
                    wino_out(Mp, wdt, d0, d1,
                             mybir.ActivationFunctionType.Identity,
                             bfin_t[:, oc:oc + 1], 0.0)
                    nc.sync.dma_start(
                        y_d[oc][:, a0 * 132:a0 * 132 + nst * 132],
                        out_sb[oc][:, a0 * 132:a0 * 132 + nst * 132])

    nc.compile()
    return nc


# --------------------------------------------------------------------------
# entry point
# --------------------------------------------------------------------------

def _get_program():
    if 'nc' not in _PROGRAM_CACHE:
        _PROGRAM_CACHE['nc'] = build_program()
    return _PROGRAM_CACHE['nc']


def kernel(_trace=False, **inputs):
    nc = _get_program()
    in_maps = [host_prep(inputs, core) for core in range(8)]
    res = run_bass_kernel_spmd(nc, in_maps, core_ids=list(range(8)),
                               trace=_trace)
    y = np.zeros((B, C, H, W), np.float32)
    for core in range(8):
        b, s = core // 2, core % 2
        yh = np.asarray(res.results[core]['y'], dtype=np.float32)
        yh = yh.reshape(2, 128, HALF, BUFC)[:, :, :, 1:65]
        y[b, :, s * HALF:(s + 1) * HALF, :] = yh.reshape(C, HALF, 64)
    if _trace:
        return y, res
    return y

